# revision 53
# baseline (speedup 1.0000x reference)
# Trainium2 Bass kernel for nn_MultiHeadedAttention_35510789604074.
#
# Math (see reference): only the DIAGONAL of softmax(q k^T / sqrt(D)) scales v:
#   out[n, h*D+d] = v[n, h*D+d] * exp(s_nn)/sum_m exp(s_nm),  s = (x Wq^T)(x Wk^T)^T / 8
# No max-subtraction is needed: scores are O(+-8), safely inside fp32 exp range.
#
# Sharding: 8 cores = 4 batches x 2 head-groups (8 heads each). Each core:
#   - transposes its x slice to xT (bf16) via PE-transpose
#   - computes qT/kT for head PAIRS stacked [128=2x64 dims, N] (full-width matmuls)
#   - scores per 128-row n-tile into PSUM, exp+row-sum fused in one ScalarE
#     activation (accum_out) -- the row sums of exp come for free
#   - diagonal via elementwise qT*kT reduced over d with a ones-matmul
#   - av = v * diag_factor, DMA'd out per (pair, n-tile)
# Output assembled host-side; returns (av, x) like the reference.
#
# ScalarE (exp) is the bound engine (~316us busy/core); the emission order is
# arranged so its pipeline starts early (m-chunk-outer scores; interleaved
# q/k projection chunks) and never waits on filler work (v / next-pair
# weights are emitted at lower priority than the scores).

import numpy as np

N_TOK = 2048
EMB = 1024
D = 64
H_LOC = 8          # heads per core
P = 128


def build_program(n_tok=N_TOK, emb=EMB, h_loc=H_LOC, num_devices=8, reps=1,
                  drop=()):
    import concourse.bass as bass
    import concourse.tile as tile
    from concourse import bacc, mybir
    from concourse.masks import make_identity

    f32 = mybir.dt.float32
    bf16 = mybir.dt.bfloat16
    i32 = mybir.dt.int32
    i16 = mybir.dt.int16
    Exp = mybir.ActivationFunctionType.Exp
    # Schraudolph exp for the DVE-offloaded denominator chunks, in bf16:
    #   exp(p * 0.125) ~= bitcast_bf16(int16(p * SCH_A16 + SCH_B16))
    # (i16 tensor_scalar + bf16 tensor_reduce run ~25% faster than the
    # i32/f32 pair; the reduce accumulates in f32 so the sum keeps ~0.2%
    # accuracy, plenty for the softmax denominator)
    SCH_A16 = 0.125 * (1 << 7) / float(np.log(2.0))
    SCH_B16 = float((127 << 7) - 482760 / (1 << 16))

    NT = n_tok // P          # n-tiles (16)
    NE = emb // P            # e-chunks (8)
    NPAIR = h_loc // 2       # head pairs (4)
    DC = h_loc * D           # local head-dim columns (512)
    NCH = n_tok // 512       # 512-wide n chunks (4)
    SC = 1024                # scores psum tile free size
    NSC = n_tok // SC        # scores chunks per row (2)
    XW = min(4, NT)          # n-tiles per x DMA

    nc = bacc.Bacc("TRN2", target_bir_lowering=False, debug=False,
                   num_devices=num_devices)
    x_in = nc.dram_tensor("x", [n_tok, emb], f32, kind="ExternalInput")
    wq_in = nc.dram_tensor("wq", [DC, emb], f32, kind="ExternalInput")
    wk_in = nc.dram_tensor("wk", [DC, emb], f32, kind="ExternalInput")
    wv_in = nc.dram_tensor("wv", [DC, emb], f32, kind="ExternalInput")
    out = nc.dram_tensor("out", [n_tok, DC], f32, kind="ExternalOutput")

    with tile.TileContext(nc) as tc:
        with (
            tc.tile_pool(name="consts", bufs=1) as consts,
            tc.tile_pool(name="persist", bufs=1) as persist,
            tc.tile_pool(name="stage", bufs=2) as stage,
            tc.tile_pool(name="work", bufs=2) as work,
            tc.tile_pool(name="ps_sc", bufs=2, space="PSUM") as ps_sc,
            tc.tile_pool(name="ps_sv", bufs=1, space="PSUM") as ps_sv,
            tc.tile_pool(name="ps_pr", bufs=1, space="PSUM") as ps_pr,
        ):
            ident = consts.tile([P, P], bf16)
            make_identity(nc, ident)
            # ones2[d, j] = 1 where head j of the pair owns dim d
            ones2 = consts.tile([P, 2], bf16)
            nc.gpsimd.memset(ones2[:, :], 0.0)
            nc.gpsimd.memset(ones2[0:64, 0:1], 1.0)
            nc.gpsimd.memset(ones2[64:128, 1:2], 1.0)

            # ---- transposes: PE-transpose 4 blocks into one psum tile, then
            # a single wide copy, alternating DVE/ScalarE so neither chain
            # bounds the startup ramp ----
            # during the startup ramp ScalarE is idle, so alternating the
            # transpose copies DVE/ScalarE halves the copy chain; once the
            # exp stream is running ScalarE is the bound engine and all
            # copies go to DVE
            _copy_alt = [0]
            _ramp = [True]

            def copy_out(dst, src):
                _copy_alt[0] ^= 1
                if _ramp[0] and _copy_alt[0]:
                    nc.scalar.copy(dst, src)
                else:
                    nc.vector.tensor_copy(dst, src)

            def transpose_4blocks(dst, srcs):
                tp = ps_pr.tile([P, XW * P], bf16, tag="tp")
                for j, src in enumerate(srcs):
                    nc.tensor.transpose(tp[:, j * P:(j + 1) * P], src, ident)
                copy_out(dst, tp[:, :len(srcs) * P])

            # x: 4-tile-wide cast-loads, transposed into xT (bf16) per group
            xT = persist.tile([P, NE, n_tok], bf16)
            x_r = x_in.rearrange("(g j p) e -> p g j e", p=P, j=XW)
            x_nats = []
            _rep = [0]  # current rep (names must stay unique across reps)

            def load_x_group(g):
                x_nat = stage.tile([P, XW, emb], bf16, tag="xnat", bufs=4,
                                   name=f"xnat{g}_r{_rep[0]}")
                if g == 0 and XW >= 2:
                    # two dma_starts for the first group so the first half's
                    # transposes can begin before the whole group lands
                    h = XW // 2
                    nc.gpsimd.dma_start(x_nat[:, :h, :], x_r[:, g, :h])
                    nc.gpsimd.dma_start(x_nat[:, h:, :], x_r[:, g, h:])
                else:
                    nc.gpsimd.dma_start(x_nat[:, :, :], x_r[:, g])
                x_nats.append(x_nat)

            def transpose_x_group(g):
                x_nat = x_nats[g]
                for ec in range(NE):
                    transpose_4blocks(
                        xT[:, ec, g * XW * P:(g + 1) * XW * P],
                        [x_nat[:, j, ec * P:(ec + 1) * P] for j in range(XW)])

            # weights: one cast-load per tensor, transposed per e-chunk
            w_nats = {}
            w_Ts = {}

            def load_w(wname, w_in):
                if wname not in w_nats:
                    w_nats[wname] = persist.tile([P, DC // P, emb], bf16,
                                                 name=f"{wname}nat")
                    w_Ts[wname] = persist.tile([P, NE, DC], bf16,
                                               name=f"{wname}T")
                nc.gpsimd.dma_start(
                    w_nats[wname][:, :, :],
                    w_in.rearrange("(d p) e -> p d e", p=P))
                return w_Ts[wname]

            def transpose_w(wname):
                w_nat, wT = w_nats[wname], w_Ts[wname]
                for ec in range(NE):
                    transpose_4blocks(
                        wT[:, ec, :],
                        [w_nat[:, dt_, ec * P:(ec + 1) * P]
                         for dt_ in range(DC // P)])

            def project_chunk(wT, tT, p_, nch):
                pq = ps_pr.tile([P, 512], f32, tag="proj")
                for ec in range(NE):
                    nc.tensor.matmul(
                        pq[:, :],
                        lhsT=wT[:, ec, p_ * P:(p_ + 1) * P],
                        rhs=xT[:, ec, nch * 512:(nch + 1) * 512],
                        start=(ec == 0), stop=(ec == NE - 1))
                nc.vector.tensor_copy(tT[:, nch * 512:(nch + 1) * 512], pq[:, :])

            def project_pair(p_):
                """qT/kT [128 = 2 heads x 64 dims, n_tok] bf16, interleaved so
                the first scores wave (k cols 0:1024, q cols 0:128) is ready
                as early as possible."""
                qT = work.tile([P, n_tok], bf16, tag="qT",
                               name=f"qT{p_}_r{_rep[0]}")
                kT = work.tile([P, n_tok], bf16, tag="kT",
                               name=f"kT{p_}_r{_rep[0]}")
                wqT, wkT = w_Ts["wq"], w_Ts["wk"]
                order = [(wqT, qT, 0), (wkT, kT, 0)]
                if NCH > 1:
                    order.append((wkT, kT, 1))
                order += [(wqT, qT, n) for n in range(1, NCH)]
                order += [(wkT, kT, n) for n in range(2, NCH)]
                for wT, tT, nch in order:
                    project_chunk(wT, tT, p_, nch)
                return qT, kT

            def diag_exp(qT, kT):
                """dexp[:, t, j] = exp(sum_d q*k / 8) for both pair heads."""
                qkprod = work.tile([P, n_tok], bf16, tag="qkprod")
                nc.vector.tensor_mul(qkprod[:, :], qT[:, :], kT[:, :])
                pdg = ps_pr.tile([P, 512], f32, tag="proj")
                for t in range(NT):
                    nc.tensor.matmul(pdg[:, 2 * t:2 * t + 2],
                                     lhsT=qkprod[:, t * P:(t + 1) * P],
                                     rhs=ones2[:, :], start=True, stop=True)
                dexp = work.tile([P, NT, 2], f32, tag="dexp")
                nc.scalar.activation(dexp[:, :, :], pdg[:, 0:2 * NT], Exp,
                                     scale=0.125)
                return dexp

            # two v buffers alternating by rep parity: without this, rep k+1's
            # v-projections (PE, in-order) would stall on rep k's last
            # epilogue reads and serialize the reps=N timing builds
            v_bufs = [persist.tile([P, NT, h_loc, D], bf16, name="v_all0")]
            if reps > 1:
                v_bufs.append(persist.tile([P, NT, h_loc, D], bf16,
                                           name="v_all1"))

            def emit_v(trange):
                v_all = v_bufs[_rep[0] % len(v_bufs)]
                for t in trange:
                    pv = ps_pr.tile([P, 512], f32, tag="proj")
                    for ec in range(NE):
                        nc.tensor.matmul(pv[:, :DC],
                                         lhsT=xT[:, ec, t * P:(t + 1) * P],
                                         rhs=wvT[:, ec, :],
                                         start=(ec == 0), stop=(ec == NE - 1))
                    nc.vector.tensor_copy(
                        v_all[:, t, :, :],
                        pv[:, :DC].rearrange("p (h d) -> p h d", h=h_loc))

            wvT = None

            def emit_body(rep_i, do_load=True):
              nonlocal wvT
              _rep[0] = rep_i
              x_nats.clear()
              if do_load:
                # DMA order: first x group, then q/k weights, then rest of x
                load_x_group(0)
                wqT = load_w("wq", wq_in)
                wkT = load_w("wk", wk_in)
                for g in range(1, NT // XW):
                    load_x_group(g)
                transpose_x_group(0)
                transpose_w("wq")
                transpose_w("wk")

              held = None
              for p_ in range(NPAIR):
                dlo = p_ * P
                if p_ == 0 and NCH == 4 and do_load:
                    # interleave remaining x-group transposes with the
                    # projection chunks that consume them
                    qT = work.tile([P, n_tok], bf16, tag="qT",
                                   name=f"qT0_r{rep_i}")
                    kT = work.tile([P, n_tok], bf16, tag="kT",
                                   name=f"kT0_r{rep_i}")
                    project_chunk(wkT, kT, 0, 0)
                    project_chunk(wqT, qT, 0, 0)
                    _ramp[0] = False
                    for g in range(1, 4):
                        transpose_x_group(g)
                        project_chunk(wkT, kT, 0, g)
                        project_chunk(wqT, qT, 0, g)
                else:
                    if p_ == 0 and do_load:
                        for g in range(1, NT // XW):
                            transpose_x_group(g)
                    qT, kT = project_pair(p_)

                _ramp[0] = False
                if p_ == 0 and do_load:
                    wvT = load_w("wv", wv_in)
                    transpose_w("wv")
                # scores + fused exp/row-sum, m-chunk outer; one v-unit per
                # 4 t's of the second m-chunk keeps PE fed in every window
                spart = work.tile([P, NT, 2, NSC], f32, tag="spart")
                dexp = None
                for c in range(NSC):
                    for t in range(NT):
                        for hh in range(2):
                            hb = 64 * hh
                            idx = t * 2 + hh + 2 * c
                            use_dve = ("allscalar" not in drop) and (
                                "alldve" in drop or idx % 4 == 0
                                or idx % 32 in (6, 22))
                            # DVE-consumed tiles rotate their own pool so a
                            # slow Schraudolph chunk never stalls the
                            # act-stream's buffer rotation
                            pool = ps_sv if (use_dve and "exp" not in drop) \
                                else ps_sc
                            ps = pool.tile([P, SC], f32, tag="scores")
                            for half in range(SC // 512):
                                mo = c * SC + half * 512
                                nc.tensor.matmul(
                                    ps[:, half * 512:(half + 1) * 512],
                                    lhsT=qT[hb:hb + 64, t * P:(t + 1) * P],
                                    rhs=kT[hb:hb + 64, mo:mo + 512],
                                    start=True, stop=True)
                            if "exp" in drop:
                                nc.vector.tensor_reduce(
                                    spart[:, t, hh, c:c + 1], ps[:, 0:8],
                                    axis=mybir.AxisListType.X,
                                    op=mybir.AluOpType.add)
                            elif use_dve:
                                # DVE Schraudolph path: ScalarE is the bound
                                # engine, so ~1/4 of the denominator chunks
                                # run as bit-trick exp + reduce on VectorE
                                icast = work.tile([P, SC], i16, tag="icast")
                                nc.vector.tensor_scalar(
                                    icast[:, :], ps[:, :], SCH_A16, SCH_B16,
                                    mybir.AluOpType.mult,
                                    mybir.AluOpType.add)
                                nc.vector.tensor_reduce(
                                    spart[:, t, hh, c:c + 1],
                                    icast.bitcast(bf16)[:, :],
                                    axis=mybir.AxisListType.X,
                                    op=mybir.AluOpType.add)
                            else:
                                # bf16 dst halves the ScalarE write cost
                                # (703ns vs 920ns per [128,1024] chunk);
                                # the exp values are only needed via accum
                                expd = work.tile([P, SC], bf16, tag="expd")
                                nc.scalar.activation(
                                    expd[:, :], ps[:, :], Exp, scale=0.125,
                                    accum_out=spart[:, t, hh, c:c + 1])
                    if c == 0:
                        # full qT/kT are complete once the first wave is
                        # emitted, so the diagonal comes for free here
                        dexp = diag_exp(qT, kT)

                def emit_epilogue(spart, dexp, dlo, trange, Fcache={}):
                    # batched: F[:, t, hh] = dexp/rowsum in 3 wide DVE ops per
                    # pair, then ONE broadcast-mul + ONE dma per t-range
                    if "epi" in drop or len(trange) == 0:
                        return
                    v_all = v_bufs[_rep[0] % len(v_bufs)]
                    key = id(spart)
                    if key not in Fcache:
                        F = work.tile([P, NT, 2], f32, tag="fcol", bufs=2,
                                      name=f"F{dlo}_r{_rep[0]}")
                        if NSC == 2:
                            nc.vector.tensor_add(F[:, :, :],
                                                 spart[:, :, :, 0],
                                                 spart[:, :, :, 1])
                            nc.vector.reciprocal(F[:, :, :], F[:, :, :])
                        else:
                            nc.vector.reciprocal(F[:, :, :],
                                                 spart[:, :, :, 0])
                        nc.vector.tensor_mul(F[:, :, :], F[:, :, :],
                                             dexp[:, :, :])
                        Fcache.clear()
                        Fcache[key] = F
                    F = Fcache[key]
                    t0, t1 = trange[0], trange[-1] + 1
                    L = t1 - t0
                    p_idx = dlo // P
                    av = work.tile([P, NT, 2, 64], f32, tag="av", bufs=2,
                                   name=f"av{dlo}_{t0}_r{_rep[0]}")
                    nc.vector.tensor_tensor(
                        av[:, t0:t1, :, :],
                        v_all[:, t0:t1, 2 * p_idx:2 * p_idx + 2, :],
                        F[:, t0:t1, :].unsqueeze(3).broadcast_to(
                            [P, L, 2, 64]),
                        op=mybir.AluOpType.mult)
                    if "outdma" not in drop:
                        nc.sync.dma_start(
                            out.rearrange("(t p) (h e) -> p t h e",
                                          p=P, e=64)[
                                :, t0:t1, 2 * p_idx:2 * p_idx + 2, :],
                            av[:, t0:t1, :, :])

                # Each pair's epilogue is split: first half in this pair's
                # window, second half early in the NEXT pair's window, so the
                # DVE load stays even and the tail doesn't pile up.
                HALF = NT // 2 if NPAIR > 1 else NT
                if p_ == 0:
                    emit_v(range(HALF))
                elif p_ == 1:
                    emit_v(range(HALF, NT))
                if held is not None:
                    emit_epilogue(*held, range(HALF, NT))
                emit_epilogue(spart, dexp, dlo, range(HALF))
                held = (spart, dexp, dlo)
                if p_ == NPAIR - 1:
                    emit_epilogue(spart, dexp, dlo, range(HALF, NT))

            for rep_i in range(reps):
                emit_body(rep_i, do_load=(rep_i == 0))

    nc.compile()
    return nc


_PROG = None


def _get_program():
    global _PROG
    if _PROG is None:
        _PROG = build_program()
    return _PROG


def kernel(x, Wq, Wk, Wv):
    from concourse.bass_utils import run_bass_kernel_spmd

    x = np.ascontiguousarray(np.asarray(x, dtype=np.float32))
    Wq = np.ascontiguousarray(np.asarray(Wq, dtype=np.float32))
    Wk = np.ascontiguousarray(np.asarray(Wk, dtype=np.float32))
    Wv = np.ascontiguousarray(np.asarray(Wv, dtype=np.float32))
    B, N, E = x.shape
    DC = H_LOC * D  # 512

    nc = _get_program()
    in_maps = []
    for c in range(8):
        b, hg = divmod(c, 2)
        in_maps.append({
            "x": x[b],
            "wq": np.ascontiguousarray(Wq[hg * DC:(hg + 1) * DC]),
            "wk": np.ascontiguousarray(Wk[hg * DC:(hg + 1) * DC]),
            "wv": np.ascontiguousarray(Wv[hg * DC:(hg + 1) * DC]),
        })
    res = run_bass_kernel_spmd(nc, in_maps, core_ids=list(range(8)))
    av = np.empty((B, N, E), np.float32)
    for c in range(8):
        b, hg = divmod(c, 2)
        av[b, :, hg * DC:(hg + 1) * DC] = res.results[c]["out"]
    return (av, x)

